# revision 1
# baseline (speedup 1.0000x reference)
"""CSWin attention kernel for 8 trn2 NeuronCores — v2 (PE tile-packing).

Layout per window (64x8 vertical stripe, S=512 tokens, C=128 channels, 4 heads
of d=32):
  - q,k,v loaded token-major [128tok, 4chunk, 128c] (one DMA per tensor),
    cast to f16, PE-transposed to [C, S] (q,k) / padded image (v)
  - QK^T: per j-chunk, 4 heads issued back-to-back as ROW-TILED matmuls
    (K=32 at tile_position=(32h,0)) -> concurrent in the PE array; outputs
    land in one [128, 2048] PSUM tile (bank h = head h)
  - exp: ONE activation per j-chunk over all 4 heads ([128,2048] PSUM->SBUF
    f16, scale folded in)
  - AV: per j-chunk, 4 heads as COL-TILED matmuls (M=32 at (0,32h)) into a
    single [128,512] PSUM bank, accumulated over j-chunks; denominators via
    4 col-tiled M=1 ones-matmuls into a second bank
  - LePE depthwise 3x3 conv as 9 diagonal-matmul taps over shifted views of
    v_pad, two 1-bank phases (+bias on the DVE copy out)
  - host does att = av/den + lepe and the window->image transpose

The walrus build in this container allows only ONE sync wait per instruction;
split_multiwaits() hoists extras onto same-engine NOPs.
"""

import numpy as np

import concourse.bass as bass
import concourse.tile as tile
from concourse import mybir
from concourse.vector_clock import ScopedClock

RES = 64
SPLIT = 8
C = 128
HEADS = 4
HD = 32
S = RES * SPLIT          # 512 tokens per window
SCALE = HD ** -0.5
B = 16
N_CORES = 8
IMGS_PER_CORE = B // N_CORES   # 2
NWIN = RES // SPLIT            # 8 windows per image
L = RES * RES                  # 4096 tokens per image
NCHUNK = S // 128              # 4 token-chunks per window

F32 = mybir.dt.float32
F32R = mybir.dt.float32r
F16 = mybir.dt.float16


# ---------------------------------------------------------------- compat ----

def _patched_drain_and_barrier(self, tick_clock, wait_clock):
    nc = self.nc
    nop_inst = nc.sync.nop(nofuse=True)
    wait_clock.add_sem_waits(nop_inst.ins, ScopedClock({None: tick_clock.global_clock}))
    si = nop_inst.ins.sync_info
    waits = list(si.on_wait) if si is not None else []
    if len(waits) > 1:
        si.on_wait = [waits[0]]
        for w in waits[1:]:
            n2 = nc.sync.nop(nofuse=True)
            n2.ins.sync_info = mybir.SyncInfo(on_wait=[w], on_update=[])
    nc.sync.drain()
    nc.all_engine_barrier()
    assert self.sems is not None
    popped = nc._tile_sem_poison_stack.pop()
    assert popped is self._sem_poison
    nc.clear_and_free_semaphores(list(self.sems.allocated().values()))
    nc.all_engine_barrier()


def _install_tile_patch():
    tile.TileContext._drain_and_barrier = _patched_drain_and_barrier


def _split_multiwaits(nc):
    """Hoist extra sync waits onto same-engine NOPs inserted just before the
    owning instruction (this walrus build allows 1 wait per instruction)."""
    for f in nc.m.functions:
        for bb in f.blocks:
            insts = bb.instructions
            if not any(
                i.sync_info is not None and len(i.sync_info.on_wait) > 1
                for i in insts
            ):
                continue
            new_insts = []
            for inst in insts:
                si = inst.sync_info
                if si is not None and len(si.on_wait) > 1:
                    waits = list(si.on_wait)
                    for w in waits[:-1]:
                        nop = mybir.InstNoOp(
                            name=nc.get_next_instruction_name(), ins=[], outs=[]
                        )
                        nop.engine = inst.engine
                        nop.sync_info = mybir.SyncInfo(on_wait=[w], on_update=[])
                        new_insts.append(nop)
                    si.on_wait = [waits[-1]]
                new_insts.append(inst)
            bb.instructions = new_insts


# ---------------------------------------------------------------- device ----

def _build_nc(n_windows=IMGS_PER_CORE * NWIN):
    _install_tile_patch()
    nc = bass.Bass(trn_type="TRN2", num_devices=N_CORES)

    q_d = nc.dram_tensor("q", [IMGS_PER_CORE, L, C], F32R, kind="ExternalInput")
    k_d = nc.dram_tensor("k", [IMGS_PER_CORE, L, C], F32R, kind="ExternalInput")
    v_d = nc.dram_tensor("v", [IMGS_PER_CORE, L, C], F32R, kind="ExternalInput")
    diag_d = nc.dram_tensor("diag", [C, 9 * C], F16, kind="ExternalInput")
    ident_d = nc.dram_tensor("ident", [C, C], F32R, kind="ExternalInput")
    bias_d = nc.dram_tensor("bias", [C, 1], F32, kind="ExternalInput")
    av_d = nc.dram_tensor(
        "avT", [IMGS_PER_CORE, NWIN, C, S], F32, kind="ExternalOutput"
    )
    den_d = nc.dram_tensor(
        "denT", [IMGS_PER_CORE, NWIN, HEADS, S], F32, kind="ExternalOutput"
    )
    lp_d = nc.dram_tensor(
        "lepeT", [IMGS_PER_CORE, NWIN, C, S], F32, kind="ExternalOutput"
    )

    # [img, y, x, c] views of DRAM tensors (per-chunk 3D DMAs; the DMA AP
    # balancer cannot handle 4D sources)
    def img_view(t):
        return t.ap().rearrange("b (y x) c -> b y x c", x=RES)

    qv, kv, vv = img_view(q_d), img_view(k_d), img_view(v_d)

    # LePE padded image: [Y=66, X=10] plus one lead cell so the interior
    # starts at an even offset: cell(y, x) = IMG0 + (y+1)*XP + (x+1)
    XP, YP = SPLIT + 2, RES + 2
    IMG0 = 1
    PADN = XP * YP + 2                   # 662 (even-sized, 1 lead + 1 tail)
    P0 = IMG0 + XP + 1                   # first interior cell = 12 (even)
    P1 = P0 + (RES - 1) * XP + SPLIT    # one past last interior = 650 (even)

    TAPS = [(0, 0)] + [
        (dy, dx) for dy in (-1, 0, 1) for dx in (-1, 0, 1) if (dy, dx) != (0, 0)
    ]
    def tap_idx(dy, dx):
        return (dy + 1) * 3 + (dx + 1)

    with tile.TileContext(nc) as tc:
        with (
            tc.tile_pool(name="const", bufs=1) as const,
            tc.tile_pool(name="nat", bufs=6) as nat,          # q/k/v token-major
            tc.tile_pool(name="tposed", bufs=4) as tposed,    # q_T/k_T [C,S]
            tc.tile_pool(name="vpadp", bufs=2) as vpadp,      # v_pad
            tc.tile_pool(name="expt", bufs=6) as expt,        # exp'd scoresT
            tc.tile_pool(name="sbout", bufs=2) as sbout,      # av/den/lepe SBUF
            tc.tile_pool(name="scoresp", bufs=2, space="PSUM") as scoresp,
            tc.tile_pool(name="avp", bufs=1, space="PSUM") as avp,
            tc.tile_pool(name="denp", bufs=1, space="PSUM") as denp,
            tc.tile_pool(name="auxp", bufs=2, space="PSUM") as auxp,
        ):
            diag_sb = const.tile([C, 9, C], F16)
            nc.sync.dma_start(out=diag_sb, in_=diag_d.ap().rearrange("c (t m) -> c t m", t=9))
            ident_b = const.tile([C, C], F16)
            nc.gpsimd.dma_start(out=ident_b, in_=ident_d.ap().bitcast(F32))
            bias_sb = const.tile([C, 1], F32)
            nc.sync.dma_start(out=bias_sb, in_=bias_d.ap())
            ones_sb = const.tile([128, 1], F16)
            nc.vector.memset(ones_sb, 1.0)

            for w in range(n_windows):
                b, sx = divmod(w, NWIN)

                # ---- load q,k,v token-major: [128, chunk, 128] --------------
                x0 = sx * SPLIT
                nats = []
                for src_i, src in enumerate((qv, kv, vv)):
                    t = nat.tile([128, NCHUNK, C], F32R, tag="nat_in")
                    eng = nc.gpsimd if src_i == 2 else nc.sync
                    for jc in range(NCHUNK):
                        eng.dma_start(
                            out=t[:, jc, :],
                            in_=src[b, 16 * jc : 16 * (jc + 1), x0 : x0 + SPLIT, :],
                        )
                    nats.append(t)
                q_nat, k_nat, v_nat = nats

                # ---- fp16 casts + PE transposes: q,k -> [C, S]; v -> padded -
                v_pad = vpadp.tile([C, PADN], F16, tag="vpad")
                nc.vector.memset(v_pad, 0.0)
                v_pad_in = bass.AP(
                    tensor=v_pad.tensor,
                    offset=v_pad.offset + P0,
                    ap=[v_pad.ap[0], [XP, RES], [1, SPLIT]],
                )  # interior cells, strided over the padded image
                tps = []
                cbs = []
                for ti_, t_nat in enumerate((q_nat, k_nat, v_nat)):
                    cb = nat.tile([128, NCHUNK, C], F16, tag="nat_b")
                    nc.vector.tensor_copy(out=cb, in_=t_nat)
                    cbs.append(cb)
                    ps = auxp.tile([128, 512], F16, tag="aux", padded_shape=[128, 1024])
                    for jc in range(NCHUNK):
                        nc.tensor.transpose(
                            ps[:, 128 * jc : 128 * (jc + 1)],
                            cb[:, jc, :],
                            ident_b,
                        )
                    if ti_ < 2:
                        tsb = tposed.tile([C, S], F16, tag="tposed")
                        nc.vector.tensor_copy(out=tsb, in_=ps)
                        tps.append(tsb)
                    else:
                        nc.vector.tensor_copy(out=v_pad_in, in_=ps)
                q_T, k_T = tps
                v_b = cbs[2]             # AV stationary (token-major f16)

                # ---- scores -> exp -> AV (+denominator), per j-chunk --------
                # One 4-bank scores tile per window; half A (banks 0-1) holds
                # heads 0,1 and half B (banks 2-3) heads 2,3. exp of half A
                # overlaps the QK writes of half B and vice versa, keeping the
                # Scalar engine continuously busy with N=1024 exp calls.
                av_ps = avp.tile([128, S], F32, tag="av")
                den_ps = denp.tile([128, S], F32, tag="den")

                def emit_av_den(jc, ets):
                    for h in range(HEADS):
                        hp = 32 * h
                        et_h = ets[h // 2][:, S * (h % 2) : S * (h % 2 + 1)]
                        nc.tensor.matmul(
                            av_ps[hp : hp + 32, :],
                            v_b[:, jc, hp : hp + 32],
                            et_h,
                            start=(jc == 0),
                            stop=(jc == NCHUNK - 1),
                            tile_position=(0, hp),
                            skip_group_check=True,
                        )
                    for h in range(HEADS):
                        hp = 32 * h
                        et_h = ets[h // 2][:, S * (h % 2) : S * (h % 2 + 1)]
                        nc.tensor.matmul(
                            den_ps[hp : hp + 1, :],
                            ones_sb,
                            et_h,
                            start=(jc == 0),
                            stop=(jc == NCHUNK - 1),
                            tile_position=(0, hp),
                            skip_group_check=True,
                        )

                prev = None
                for jc in range(NCHUNK):
                    ets = []
                    for half in range(2):
                        st = scoresp.tile([128, 2 * S], F32, tag="scores")
                        for hh in range(2):
                            h = 2 * half + hh
                            hp = 32 * h
                            nc.tensor.matmul(
                                st[:, S * hh : S * (hh + 1)],
                                k_T[hp : hp + 32, 128 * jc : 128 * (jc + 1)],
                                q_T[hp : hp + 32, :],
                                start=True,
                                stop=True,
                                tile_position=(hp, 0),
                            )
                        et = expt.tile([128, 2 * S], F16, tag="expt")
                        nc.scalar.activation(
                            out=et,
                            in_=st,
                            func=mybir.ActivationFunctionType.Exp,
                            scale=float(SCALE),
                        )
                        ets.append(et)
                    if prev is not None:
                        emit_av_den(*prev)
                    prev = (jc, ets)
                emit_av_den(*prev)

                av_sb = sbout.tile([128, S], F32, tag="av_sb")
                nc.vector.tensor_copy(out=av_sb, in_=av_ps)
                nc.gpsimd.dma_start(out=av_d.ap()[b, sx], in_=av_sb)
                den_sb = sbout.tile([128, S], F32, tag="den_sb")
                nc.vector.tensor_copy(out=den_sb, in_=den_ps)
                for h in range(HEADS):
                    nc.gpsimd.dma_start(
                        out=den_d.ap()[b, sx, h : h + 1, :],
                        in_=den_sb[32 * h : 32 * h + 1, :],
                    )

                # ---- LePE: 9 diagonal-matmul taps, two 1-bank phases --------
                sb_lepe = sbout.tile([128, S], F32, tag="sb_lepe")
                YSPL = (512 - P0) // XP        # 50 full y-rows fit in bank A
                for phase in range(2):
                    base = 0 if phase == 0 else 512
                    lo = P0 if phase == 0 else 512
                    hi = 512 if phase == 0 else P1
                    lt = auxp.tile([128, 512], F32, tag="aux")
                    for (dy, dx) in TAPS:
                        ti = tap_idx(dy, dx)
                        d = XP * dy + dx
                        nc.tensor.matmul(
                            lt[:, lo - base : hi - base],
                            diag_sb[:, ti, :],
                            v_pad[:, lo + d : hi + d],
                            start=(dy == 0 and dx == 0), stop=(ti == 8),
                            skip_group_check=True,
                        )
                    if phase == 0:
                        ys, ny = 0, YSPL
                    else:
                        ys, ny = YSPL, RES - YSPL
                    lepe_in = bass.AP(
                        tensor=lt.tensor,
                        offset=lt.offset + (P0 + ys * XP - base),
                        ap=[lt.ap[0], [XP, ny], [1, SPLIT]],
                    )
                    nc.vector.tensor_scalar(
                        out=sb_lepe[:, ys * SPLIT : (ys + ny) * SPLIT],
                        in0=lepe_in, scalar1=bias_sb, scalar2=None,
                        op0=mybir.AluOpType.add,
                    )
                nc.gpsimd.dma_start(out=lp_d.ap()[b, sx], in_=sb_lepe)

    _split_multiwaits(nc)
    return nc


# ------------------------------------------------------------------ host ----

_NC_CACHE = {}


def _get_nc(n_windows):
    key = n_windows
    if key not in _NC_CACHE:
        _NC_CACHE[key] = _build_nc(n_windows)
    return _NC_CACHE[key]


def _host_consts(conv_w, conv_b):
    # diag[c, t*C + m] = (c==m) * conv_w[c, 0, dy, dx],  t = (dy+1)*3+(dx+1)
    w = conv_w.reshape(C, 9).astype(np.float32)
    diag = np.zeros((C, 9, C), dtype=np.float32)
    idx = np.arange(C)
    for t in range(9):
        diag[idx, t, idx] = w[:, t]
    ident = np.eye(C, dtype=np.float32)
    bias = conv_b.reshape(C, 1).astype(np.float32)
    return diag.reshape(C, 9 * C).astype(np.float16), ident, bias


def kernel(qkv, conv_w, conv_b):
    from concourse.bass_utils import run_bass_kernel_spmd

    qkv = np.asarray(qkv, dtype=np.float32)
    diag, ident, bias = _host_consts(
        np.asarray(conv_w, np.float32), np.asarray(conv_b, np.float32)
    )
    nc = _get_nc(IMGS_PER_CORE * NWIN)

    in_maps = []
    for core in range(N_CORES):
        bs = slice(core * IMGS_PER_CORE, (core + 1) * IMGS_PER_CORE)
        in_maps.append(
            {
                "q": np.ascontiguousarray(qkv[0, bs]),
                "k": np.ascontiguousarray(qkv[1, bs]),
                "v": np.ascontiguousarray(qkv[2, bs]),
                "diag": diag,
                "ident": ident,
                "bias": bias,
            }
        )

    res = run_bass_kernel_spmd(nc, in_maps, core_ids=list(range(N_CORES)))
    global LAST_RESULT
    LAST_RESULT = res
    outs = []
    for r in res.results:
        av = r["avT"]              # [2, 8, 128, 512]
        den = r["denT"]            # [2, 8, 4, 512]
        lp = r["lepeT"]            # [2, 8, 128, 512]
        att = av.reshape(IMGS_PER_CORE, NWIN, HEADS, HD, S) / den.reshape(
            IMGS_PER_CORE, NWIN, HEADS, 1, S
        )
        o = att.reshape(IMGS_PER_CORE, NWIN, C, S) + lp
        o = o.reshape(IMGS_PER_CORE, NWIN, C, RES, SPLIT)
        o = o.transpose(0, 3, 1, 4, 2).reshape(IMGS_PER_CORE, RES, RES, C)
        outs.append(o)
    return np.concatenate(outs, axis=0)


LAST_RESULT = None



# revision 9
# speedup vs baseline: 1.2971x; 1.2971x over previous
"""CSWin attention kernel for 8 trn2 NeuronCores — v3.

Strategy vs v2 (292us baseline):
  - host pre-casts qkv to f16 AND pre-transposes per window:
      qT/kT   [c=128, win, s=512]   (channel-major, contiguous big DMAs)
      vtok    [tok128, win, jc, c]  (token-major for AV stationary)
      vpad    [c, win, 662]         (host-padded LePE image, zero ring)
    -> no on-device casts, no PE transposes, 4-5 big contiguous input
       DMAs instead of 192 strided per-chunk DMAs (was ~118us of
       engine trigger time).
  - QK^T per chunk-half into PSUM f32 [128,1024] (2 heads row-tiled).
  - exp split across engines: ScalarE activation(Exp) for 5 of 8
    half-tiles per window, DVE Schraudolph bit-trick (i16 = s*A+B
    viewed as f16) for the other 3 -> breaks the 131us ScalarE floor.
  - AV + den(ones-matmul) accumulated over chunks in PSUM col-tiled.
  - LePE 3x3 depthwise conv: 9 diagonal-matmul taps with STRIDED
    moving view (exactly 512 interior cols, one phase, one PSUM bank);
    bias added on host.
  - outputs copied PSUM->SBUF f16 (DVE/ScalarE; GPSIMD can't touch
    PSUM) and DMA'd f16; host does att = av/den + lepe + bias and the
    window->image transpose.

PSUM budget: scores 2x[128,1024](4 banks) + av(1) + den(1) + lepe(2) = 8.
"""

import numpy as np

import concourse.bass as bass
import concourse.tile as tile
from concourse import mybir
from concourse.vector_clock import ScopedClock

RES = 64
SPLIT = 8
C = 128
HEADS = 4
HD = 32
S = RES * SPLIT          # 512 tokens per window
SCALE = HD ** -0.5
B = 16
N_CORES = 8
IMGS_PER_CORE = B // N_CORES   # 2
NWIN_IMG = RES // SPLIT        # 8 windows per image
NW = IMGS_PER_CORE * NWIN_IMG  # 16 windows per core
NCHUNK = S // 128              # 4 token-chunks per window

F32 = mybir.dt.float32
F16 = mybir.dt.float16
I16 = mybir.dt.int16

LOG2E = 1.4426950408889634
# Schraudolph f16: i16 = s * A + B, bit pattern of ~exp(s*SCALE)
SCH_A = float(SCALE * LOG2E * 1024.0)
SCH_B = float(15 * 1024 - 45 + 0.5)

# LePE padded image geometry (host-built): cell(y,x) = 12 + y*10 + x
XP = SPLIT + 2                 # 10
PADN = XP * (RES + 2) + 2      # 662
P0 = 12

# which half-tiles (idx = 2*jc + half, 0..7) use the DVE Schraudolph exp
DVE_HALVES = frozenset({1, 4, 6})
TAPS = [(dy, dx) for dy in (-1, 0, 1) for dx in (-1, 0, 1)]


# ---------------------------------------------------------------- compat ----

def _patched_drain_and_barrier(self, tick_clock, wait_clock):
    nc = self.nc
    nop_inst = nc.sync.nop(nofuse=True)
    wait_clock.add_sem_waits(nop_inst.ins, ScopedClock({None: tick_clock.global_clock}))
    si = nop_inst.ins.sync_info
    waits = list(si.on_wait) if si is not None else []
    if len(waits) > 1:
        si.on_wait = [waits[0]]
        for w in waits[1:]:
            n2 = nc.sync.nop(nofuse=True)
            n2.ins.sync_info = mybir.SyncInfo(on_wait=[w], on_update=[])
    nc.sync.drain()
    nc.all_engine_barrier()
    assert self.sems is not None
    popped = nc._tile_sem_poison_stack.pop()
    assert popped is self._sem_poison
    nc.clear_and_free_semaphores(list(self.sems.allocated().values()))
    nc.all_engine_barrier()


def _install_tile_patch():
    tile.TileContext._drain_and_barrier = _patched_drain_and_barrier


def _split_multiwaits(nc):
    """Hoist extra sync waits onto same-engine NOPs inserted just before the
    owning instruction (this walrus build allows 1 wait per instruction)."""
    for f in nc.m.functions:
        for bb in f.blocks:
            insts = bb.instructions
            if not any(
                i.sync_info is not None and len(i.sync_info.on_wait) > 1
                for i in insts
            ):
                continue
            new_insts = []
            for inst in insts:
                si = inst.sync_info
                if si is not None and len(si.on_wait) > 1:
                    waits = list(si.on_wait)
                    for w in waits[:-1]:
                        nop = mybir.InstNoOp(
                            name=nc.get_next_instruction_name(), ins=[], outs=[]
                        )
                        nop.engine = inst.engine
                        nop.sync_info = mybir.SyncInfo(on_wait=[w], on_update=[])
                        new_insts.append(nop)
                    si.on_wait = [waits[-1]]
                new_insts.append(inst)
            bb.instructions = new_insts


# ---------------------------------------------------------------- device ----

def _build_nc():
    _install_tile_patch()
    nc = bass.Bass(trn_type="TRN2", num_devices=N_CORES)

    qT_d = nc.dram_tensor("qT", [C, NW * S], F16, kind="ExternalInput")
    kT_d = nc.dram_tensor("kT", [C, NW * S], F16, kind="ExternalInput")
    vtok_d = nc.dram_tensor("vtok", [128, NW * NCHUNK * C], F16, kind="ExternalInput")
    vpad_d = nc.dram_tensor("vpad", [C, NW * PADN], F16, kind="ExternalInput")
    diag_d = nc.dram_tensor("diag", [C, 9 * C], F16, kind="ExternalInput")

    av_d = nc.dram_tensor("avT", [NW, C, S], F16, kind="ExternalOutput")
    den_d = nc.dram_tensor("denT", [NW, HEADS, S], F16, kind="ExternalOutput")
    lp_d = nc.dram_tensor("lepeT", [NW, C, S], F16, kind="ExternalOutput")

    GRP = 4  # windows per input-DMA group

    with tile.TileContext(nc) as tc:
        with (
            tc.tile_pool(name="const", bufs=1) as const,
            tc.tile_pool(name="inp", bufs=1) as inp,
            tc.tile_pool(name="expt", bufs=4) as expt,
            tc.tile_pool(name="sbout", bufs=2) as sbout,
            tc.tile_pool(name="scoresp", bufs=2, space="PSUM") as scoresp,
            tc.tile_pool(name="avp", bufs=1, space="PSUM") as avp,
            tc.tile_pool(name="denp", bufs=1, space="PSUM") as denp,
            tc.tile_pool(name="lepep", bufs=2, space="PSUM") as lepep,
        ):
            diag_sb = const.tile([C, 9, C], F16)
            nc.sync.dma_start(
                out=diag_sb, in_=diag_d.ap().rearrange("c (t m) -> c t m", t=9)
            )
            ones_sb = const.tile([128, 1], F16)
            nc.vector.memset(ones_sb, 1.0)

            qT_sb = inp.tile([C, NW * S], F16)
            kT_sb = inp.tile([C, NW * S], F16)
            vtok_sb = inp.tile([128, NW * NCHUNK * C], F16)
            vpad_sb = inp.tile([C, NW * PADN], F16)
            for g in range(NW // GRP):
                for t_sb, t_d, width in (
                    (qT_sb, qT_d, S),
                    (kT_sb, kT_d, S),
                    (vtok_sb, vtok_d, NCHUNK * C),
                    (vpad_sb, vpad_d, PADN),
                ):
                    lo, hi = g * GRP * width, (g + 1) * GRP * width
                    nc.sync.dma_start(out=t_sb[:, lo:hi], in_=t_d.ap()[:, lo:hi])

            for w in range(NW):
                av_ps = avp.tile([128, S], F32, tag="av")
                den_ps = denp.tile([128, S], F32, tag="den")
                ets = {}

                def qk_exp(jc, half, w=w, ets=None):
                    st = scoresp.tile([128, 2 * S], F32, tag="st")
                    for hh in range(2):
                        h = 2 * half + hh
                        hp = 32 * h
                        nc.tensor.matmul(
                            st[:, S * hh : S * (hh + 1)],
                            kT_sb[hp : hp + 32, w * S + 128 * jc : w * S + 128 * (jc + 1)],
                            qT_sb[hp : hp + 32, w * S : (w + 1) * S],
                            start=True,
                            stop=True,
                            tile_position=(hp, 0),
                        )
                    et = expt.tile([128, 2 * S], F16, tag="et")
                    idx = 2 * jc + half
                    if idx in DVE_HALVES:
                        nc.vector.tensor_scalar(
                            out=et.bitcast(I16),
                            in0=st,
                            scalar1=SCH_A,
                            scalar2=SCH_B,
                            op0=mybir.AluOpType.mult,
                            op1=mybir.AluOpType.add,
                        )
                    else:
                        nc.scalar.activation(
                            out=et,
                            in_=st,
                            func=mybir.ActivationFunctionType.Exp,
                            scale=float(SCALE),
                        )
                    ets[idx] = et

                def av_den(jc, w=w, ets=None):
                    for h in range(HEADS):
                        hp = 32 * h
                        et_h = ets[2 * jc + h // 2][:, S * (h % 2) : S * (h % 2 + 1)]
                        nc.tensor.matmul(
                            av_ps[hp : hp + 32, :],
                            vtok_sb[:, w * S + jc * C + hp : w * S + jc * C + hp + 32],
                            et_h,
                            start=(jc == 0),
                            stop=(jc == NCHUNK - 1),
                            tile_position=(0, hp),
                            skip_group_check=True,
                        )
                    for h in range(HEADS):
                        hp = 32 * h
                        et_h = ets[2 * jc + h // 2][:, S * (h % 2) : S * (h % 2 + 1)]
                        nc.tensor.matmul(
                            den_ps[hp : hp + 1, :],
                            ones_sb,
                            et_h,
                            start=(jc == 0),
                            stop=(jc == NCHUNK - 1),
                            tile_position=(0, hp),
                            skip_group_check=True,
                        )

                qk_exp(0, 0, ets=ets)
                qk_exp(0, 1, ets=ets)
                qk_exp(1, 0, ets=ets)
                qk_exp(1, 1, ets=ets)
                av_den(0, ets=ets)
                qk_exp(2, 0, ets=ets)
                qk_exp(2, 1, ets=ets)
                av_den(1, ets=ets)
                qk_exp(3, 0, ets=ets)
                qk_exp(3, 1, ets=ets)
                av_den(2, ets=ets)

                # ---- LePE taps on the PE while exp(3,*) runs ---------------
                lp = lepep.tile([128, S], F32, tag="lp")

                def vpad_view(t):
                    dy, dx = TAPS[t]
                    d = XP * dy + dx
                    return bass.AP(
                        tensor=vpad_sb.tensor,
                        offset=vpad_sb.offset + w * PADN + P0 + d,
                        ap=[vpad_sb.ap[0], [XP, RES], [1, SPLIT]],
                    )

                for t in range(9):
                    nc.tensor.matmul(
                        lp,
                        diag_sb[:, t, :],
                        vpad_view(t),
                        start=(t == 0),
                        stop=(t == 8),
                        skip_group_check=True,
                    )

                av_den(3, ets=ets)

                # ---- drain PSUM -> SBUF f16 -> DRAM ------------------------
                av_sb = sbout.tile([128, S], F16, tag="av_sb")
                nc.vector.tensor_copy(out=av_sb, in_=av_ps)
                nc.gpsimd.dma_start(out=av_d.ap()[w], in_=av_sb)

                den_sb = sbout.tile([128, S], F16, tag="den_sb")
                nc.scalar.activation(
                    out=den_sb,
                    in_=den_ps,
                    func=mybir.ActivationFunctionType.Copy,
                )
                den_view = bass.AP(
                    tensor=den_sb.tensor,
                    offset=den_sb.offset,
                    ap=[[den_sb.ap[0][0] * 32, 4], [1, S]],
                )
                nc.gpsimd.dma_start(out=den_d.ap()[w], in_=den_view)

                lp_sb = sbout.tile([128, S], F16, tag="lp_sb")
                nc.vector.tensor_copy(out=lp_sb, in_=lp)
                nc.gpsimd.dma_start(out=lp_d.ap()[w], in_=lp_sb)

    _split_multiwaits(nc)
    return nc


# ------------------------------------------------------------------ host ----

_NC_CACHE = {}


def _get_nc():
    if "nc" not in _NC_CACHE:
        _NC_CACHE["nc"] = _build_nc()
    return _NC_CACHE["nc"]


def _host_prep(qkv, conv_w):
    """Build per-core input arrays (all f16)."""
    f16 = np.float16
    # [3, B, 4096, 128] -> window grids [3, B, y, sx, x, c]
    qkv_w = qkv.reshape(3, B, RES, NWIN_IMG, SPLIT, C)

    cores = []
    for core in range(N_CORES):
        bs = slice(core * IMGS_PER_CORE, (core + 1) * IMGS_PER_CORE)
        q = qkv_w[0, bs]   # [2, y, sx, x, c]
        k = qkv_w[1, bs]
        v = qkv_w[2, bs]

        # [c, img, sx, y, x] -> [128, NW*512]
        qT = np.ascontiguousarray(q.transpose(4, 0, 2, 1, 3)).reshape(C, NW * S)
        kT = np.ascontiguousarray(k.transpose(4, 0, 2, 1, 3)).reshape(C, NW * S)

        # v token-major: [yy, x, img, sx, jc, c] -> [128, NW*4*128]
        vt = v.reshape(IMGS_PER_CORE, NCHUNK, 16, NWIN_IMG, SPLIT, C)
        vtok = np.ascontiguousarray(vt.transpose(2, 4, 0, 3, 1, 5)).reshape(
            128, NW * NCHUNK * C
        )

        # vpad: [c, win, 662] with interior at 12 + y*10 + x
        vimg = np.ascontiguousarray(v.transpose(4, 0, 2, 1, 3))  # [c, img, sx, y, x]
        vpad = np.zeros((C, NW, PADN), dtype=f16)
        vpad_v = vpad[:, :, 1:661].reshape(C, NW, RES + 2, XP)
        vpad_v[:, :, 1:-1, 1:-1] = vimg.reshape(C, NW, RES, SPLIT)

        cores.append(
            {
                "qT": qT.astype(f16),
                "kT": kT.astype(f16),
                "vtok": vtok.astype(f16),
                "vpad": vpad.reshape(C, NW * PADN),
            }
        )

    w9 = conv_w.reshape(C, 9).astype(np.float32)
    diag = np.zeros((C, 9, C), dtype=np.float32)
    idx = np.arange(C)
    for t in range(9):
        diag[idx, t, idx] = w9[:, t]
    diag = diag.reshape(C, 9 * C).astype(f16)
    for m in cores:
        m["diag"] = diag
    return cores


def kernel(qkv, conv_w, conv_b):
    from concourse.bass_utils import run_bass_kernel_spmd

    qkv = np.asarray(qkv, dtype=np.float32)
    conv_w = np.asarray(conv_w, np.float32)
    conv_b = np.asarray(conv_b, np.float32)

    nc = _get_nc()
    in_maps = _host_prep(qkv, conv_w)

    res = run_bass_kernel_spmd(nc, in_maps, core_ids=list(range(N_CORES)))
    global LAST_RESULT
    LAST_RESULT = res

    outs = []
    for r in res.results:
        av = r["avT"].astype(np.float32)       # [16, 128, 512]
        den = r["denT"].astype(np.float32)     # [16, 4, 512]
        lp = r["lepeT"].astype(np.float32)     # [16, 128, 512]
        att = av.reshape(NW, HEADS, HD, S) / den.reshape(NW, HEADS, 1, S)
        o = att.reshape(NW, C, S) + lp + conv_b.astype(np.float32)[None, :, None]
        # [win, c, s] -> [img, y, x, c]
        o = o.reshape(IMGS_PER_CORE, NWIN_IMG, C, RES, SPLIT)
        o = o.transpose(0, 3, 1, 4, 2).reshape(IMGS_PER_CORE, RES, RES, C)
        outs.append(o)
    return np.concatenate(outs, axis=0)


LAST_RESULT = None


# revision 19
# speedup vs baseline: 1.3204x; 1.0180x over previous
"""CSWin attention kernel for 8 trn2 NeuronCores — v3.

Strategy vs v2 (292us baseline):
  - host pre-casts qkv to f16 AND pre-transposes per window:
      qT/kT   [c=128, win, s=512]   (channel-major, contiguous big DMAs)
      vtok    [tok128, win, jc, c]  (token-major for AV stationary)
      vpad    [c, win, 662]         (host-padded LePE image, zero ring)
    -> no on-device casts, no PE transposes, 4-5 big contiguous input
       DMAs instead of 192 strided per-chunk DMAs (was ~118us of
       engine trigger time).
  - QK^T per chunk-half into PSUM f32 [128,1024] (2 heads row-tiled).
  - exp split across engines: ScalarE activation(Exp) for 5 of 8
    half-tiles per window, DVE Schraudolph bit-trick (i16 = s*A+B
    viewed as f16) for the other 3 -> breaks the 131us ScalarE floor.
  - AV + den(ones-matmul) accumulated over chunks in PSUM col-tiled.
  - LePE 3x3 depthwise conv: 9 diagonal-matmul taps with STRIDED
    moving view (exactly 512 interior cols, one phase, one PSUM bank);
    bias added on host.
  - outputs copied PSUM->SBUF f16 (DVE/ScalarE; GPSIMD can't touch
    PSUM) and DMA'd f16; host does att = av/den + lepe + bias and the
    window->image transpose.

PSUM budget: scores 2x[128,1024](4 banks) + av(1) + den(1) + lepe(2) = 8.
"""

import numpy as np

import concourse.bass as bass
import concourse.tile as tile
from concourse import mybir
from concourse.vector_clock import ScopedClock

RES = 64
SPLIT = 8
C = 128
HEADS = 4
HD = 32
S = RES * SPLIT          # 512 tokens per window
SCALE = HD ** -0.5
B = 16
N_CORES = 8
IMGS_PER_CORE = B // N_CORES   # 2
NWIN_IMG = RES // SPLIT        # 8 windows per image
NW = IMGS_PER_CORE * NWIN_IMG  # 16 windows per core
NCHUNK = S // 128              # 4 token-chunks per window

F32 = mybir.dt.float32
F16 = mybir.dt.float16
I16 = mybir.dt.int16

LOG2E = 1.4426950408889634
# Schraudolph f16: i16 = s * A + B, bit pattern of ~exp(s*SCALE)
SCH_A = float(SCALE * LOG2E * 1024.0)
SCH_B = float(15 * 1024 - 45 + 0.5)

# LePE padded image geometry (host-built): cell(y,x) = 12 + y*10 + x
XP = SPLIT + 2                 # 10
PADN = XP * (RES + 2) + 2      # 662
P0 = 12

# which half-tiles (idx = 2*jc + half, 0..7) use the DVE Schraudolph exp
DVE_HALVES = frozenset({1, 5})
TAPS = [(dy, dx) for dy in (-1, 0, 1) for dx in (-1, 0, 1)]
# LePE taps computed on the DVE (scalar_tensor_tensor chain) vs the PE
DVE_TAPS = (0, 1, 2)
PE_TAPS = tuple(t for t in range(9) if t not in DVE_TAPS)


# ---------------------------------------------------------------- compat ----

def _patched_drain_and_barrier(self, tick_clock, wait_clock):
    nc = self.nc
    nop_inst = nc.sync.nop(nofuse=True)
    wait_clock.add_sem_waits(nop_inst.ins, ScopedClock({None: tick_clock.global_clock}))
    si = nop_inst.ins.sync_info
    waits = list(si.on_wait) if si is not None else []
    if len(waits) > 1:
        si.on_wait = [waits[0]]
        for w in waits[1:]:
            n2 = nc.sync.nop(nofuse=True)
            n2.ins.sync_info = mybir.SyncInfo(on_wait=[w], on_update=[])
    nc.sync.drain()
    nc.all_engine_barrier()
    assert self.sems is not None
    popped = nc._tile_sem_poison_stack.pop()
    assert popped is self._sem_poison
    nc.clear_and_free_semaphores(list(self.sems.allocated().values()))
    nc.all_engine_barrier()


def _install_tile_patch():
    tile.TileContext._drain_and_barrier = _patched_drain_and_barrier


def _split_multiwaits(nc):
    """Hoist extra sync waits onto same-engine NOPs inserted just before the
    owning instruction (this walrus build allows 1 wait per instruction)."""
    for f in nc.m.functions:
        for bb in f.blocks:
            insts = bb.instructions
            if not any(
                i.sync_info is not None and len(i.sync_info.on_wait) > 1
                for i in insts
            ):
                continue
            new_insts = []
            for inst in insts:
                si = inst.sync_info
                if si is not None and len(si.on_wait) > 1:
                    waits = list(si.on_wait)
                    for w in waits[:-1]:
                        nop = mybir.InstNoOp(
                            name=nc.get_next_instruction_name(), ins=[], outs=[]
                        )
                        nop.engine = inst.engine
                        nop.sync_info = mybir.SyncInfo(on_wait=[w], on_update=[])
                        new_insts.append(nop)
                    si.on_wait = [waits[-1]]
                new_insts.append(inst)
            bb.instructions = new_insts


# ---------------------------------------------------------------- device ----

def _build_nc():
    _install_tile_patch()
    nc = bass.Bass(trn_type="TRN2", num_devices=N_CORES)

    qT_d = nc.dram_tensor("qT", [C, NW * S], F16, kind="ExternalInput")
    kT_d = nc.dram_tensor("kT", [C, NW * S], F16, kind="ExternalInput")
    vtok_d = nc.dram_tensor("vtok", [128, NW * NCHUNK * C], F16, kind="ExternalInput")
    vpad_d = nc.dram_tensor("vpad", [C, NW * PADN], F16, kind="ExternalInput")
    diag_d = nc.dram_tensor("diag", [C, 9 * C], F16, kind="ExternalInput")

    av_d = nc.dram_tensor("avT", [NW, C, S], F16, kind="ExternalOutput")
    den_d = nc.dram_tensor("denT", [NW, HEADS, S], F16, kind="ExternalOutput")
    lp_d = nc.dram_tensor("lepeT", [NW, C, S], F16, kind="ExternalOutput")

    wtap_d = nc.dram_tensor("wtap", [C, 9], F32, kind="ExternalInput")

    GRP = 4  # windows per input-DMA group

    with tile.TileContext(nc) as tc:
        with (
            tc.tile_pool(name="const", bufs=1) as const,
            tc.tile_pool(name="inp", bufs=1) as inp,
            tc.tile_pool(name="expt", bufs=4) as expt,
            tc.tile_pool(name="sbout", bufs=2) as sbout,
            tc.tile_pool(name="lacc", bufs=2) as lacc,
            tc.tile_pool(name="scoresp", bufs=2, space="PSUM") as scoresp,
            tc.tile_pool(name="avp", bufs=2, space="PSUM") as avp,
            tc.tile_pool(name="denp", bufs=1, space="PSUM") as denp,
            tc.tile_pool(name="lepep", bufs=1, space="PSUM") as lepep,
        ):
            diag_sb = const.tile([C, 9, C], F16)
            nc.sync.dma_start(
                out=diag_sb, in_=diag_d.ap().rearrange("c (t m) -> c t m", t=9)
            )
            ones_sb = const.tile([128, 1], F16)
            nc.vector.memset(ones_sb, 1.0)
            wtap_sb = const.tile([C, 9], F32)
            nc.sync.dma_start(out=wtap_sb, in_=wtap_d.ap())

            qT_sb = inp.tile([C, NW * S], F16)
            kT_sb = inp.tile([C, NW * S], F16)
            vtok_sb = inp.tile([128, NW * NCHUNK * C], F16)
            vpad_sb = inp.tile([C, NW * PADN], F16)
            for g in range(NW // GRP):
                for t_sb, t_d, width in (
                    (qT_sb, qT_d, S),
                    (kT_sb, kT_d, S),
                    (vtok_sb, vtok_d, NCHUNK * C),
                    (vpad_sb, vpad_d, PADN),
                ):
                    lo, hi = g * GRP * width, (g + 1) * GRP * width
                    nc.sync.dma_start(out=t_sb[:, lo:hi], in_=t_d.ap()[:, lo:hi])

            def vpad_view(w, t):
                dy, dx = TAPS[t]
                d = XP * dy + dx
                return bass.AP(
                    tensor=vpad_sb.tensor,
                    offset=vpad_sb.offset + w * PADN + P0 + d,
                    ap=[vpad_sb.ap[0], [XP, RES], [1, SPLIT]],
                )

            def lepe_taps(w):
                """PE diag-matmul taps into one PSUM bank + DVE STT taps into
                an SBUF f16 accumulator; returns (lp_psum, acc_sbuf)."""
                lp = lepep.tile([128, S], F32, tag="lp")
                for i, t in enumerate(PE_TAPS):
                    nc.tensor.matmul(
                        lp,
                        diag_sb[:, t, :],
                        vpad_view(w, t),
                        start=(i == 0),
                        stop=(i == len(PE_TAPS) - 1),
                        skip_group_check=True,
                    )
                acc = lacc.tile([128, S], F16, tag="lacc")
                for i, t in enumerate(DVE_TAPS):
                    if i == 0:
                        nc.vector.tensor_scalar(
                            out=acc,
                            in0=vpad_view(w, t),
                            scalar1=wtap_sb[:, t : t + 1],
                            scalar2=None,
                            op0=mybir.AluOpType.mult,
                        )
                    else:
                        nc.vector.scalar_tensor_tensor(
                            out=acc,
                            in0=vpad_view(w, t),
                            scalar=wtap_sb[:, t : t + 1],
                            in1=acc,
                            op0=mybir.AluOpType.mult,
                            op1=mybir.AluOpType.add,
                        )
                return lp, acc

            def lepe_out(w, lp, acc):
                # lepe = PE part (PSUM f32) + DVE part (SBUF f16)
                lp_sb = sbout.tile([128, S], F16, tag="lp_sb")
                nc.vector.scalar_tensor_tensor(
                    out=lp_sb,
                    in0=lp,
                    scalar=1.0,
                    in1=acc,
                    op0=mybir.AluOpType.mult,
                    op1=mybir.AluOpType.add,
                )
                nc.gpsimd.dma_start(out=lp_d.ap()[w], in_=lp_sb)

            prev_lp = None  # (w, lp tile, acc tile) of the previous window

            for w in range(NW):
                av_ps = avp.tile([128, S], F32, tag="av")
                den_ps = denp.tile([128, S], F32, tag="den")
                ets = {}

                def qk_exp(jc, half, w=w, ets=None):
                    st = scoresp.tile([128, 2 * S], F32, tag="st")
                    for hh in range(2):
                        h = 2 * half + hh
                        hp = 32 * h
                        nc.tensor.matmul(
                            st[:, S * hh : S * (hh + 1)],
                            kT_sb[hp : hp + 32, w * S + 128 * jc : w * S + 128 * (jc + 1)],
                            qT_sb[hp : hp + 32, w * S : (w + 1) * S],
                            start=True,
                            stop=True,
                            tile_position=(hp, 0),
                        )
                    et = expt.tile([128, 2 * S], F16, tag="et")
                    idx = 2 * jc + half
                    if idx in DVE_HALVES:
                        nc.vector.tensor_scalar(
                            out=et.bitcast(I16),
                            in0=st,
                            scalar1=SCH_A,
                            scalar2=SCH_B,
                            op0=mybir.AluOpType.mult,
                            op1=mybir.AluOpType.add,
                        )
                    else:
                        nc.scalar.activation(
                            out=et,
                            in_=st,
                            func=mybir.ActivationFunctionType.Exp,
                            scale=float(SCALE),
                        )
                    ets[idx] = et

                def av_den(jc, w=w, ets=None):
                    for h in range(HEADS):
                        hp = 32 * h
                        et_h = ets[2 * jc + h // 2][:, S * (h % 2) : S * (h % 2 + 1)]
                        nc.tensor.matmul(
                            av_ps[hp : hp + 32, :],
                            vtok_sb[:, w * S + jc * C + hp : w * S + jc * C + hp + 32],
                            et_h,
                            start=(jc == 0),
                            stop=(jc == NCHUNK - 1),
                            tile_position=(0, hp),
                            skip_group_check=True,
                        )
                    for h in range(HEADS):
                        hp = 32 * h
                        et_h = ets[2 * jc + h // 2][:, S * (h % 2) : S * (h % 2 + 1)]
                        nc.tensor.matmul(
                            den_ps[hp : hp + 1, :],
                            ones_sb,
                            et_h,
                            start=(jc == 0),
                            stop=(jc == NCHUNK - 1),
                            tile_position=(0, hp),
                            skip_group_check=True,
                        )

                qk_exp(0, 0, ets=ets)
                qk_exp(0, 1, ets=ets)
                qk_exp(1, 0, ets=ets)
                qk_exp(1, 1, ets=ets)
                av_den(0, ets=ets)
                qk_exp(2, 0, ets=ets)
                qk_exp(2, 1, ets=ets)
                av_den(1, ets=ets)
                qk_exp(3, 0, ets=ets)
                qk_exp(3, 1, ets=ets)
                av_den(2, ets=ets)

                # drain previous window's lepe bank, then fill it for w;
                # the taps cover the PE while exp(3,*) drains
                if prev_lp is not None:
                    lepe_out(*prev_lp)
                prev_lp = (w, *lepe_taps(w))

                av_den(3, ets=ets)

                # ---- drain PSUM -> SBUF f16 -> DRAM (den first: bufs=1) ----
                den_sb = sbout.tile([128, S], F16, tag="den_sb")
                nc.vector.tensor_copy(out=den_sb, in_=den_ps)
                den_view = bass.AP(
                    tensor=den_sb.tensor,
                    offset=den_sb.offset,
                    ap=[[den_sb.ap[0][0] * 32, 4], [1, S]],
                )
                nc.gpsimd.dma_start(out=den_d.ap()[w], in_=den_view)

                av_sb = sbout.tile([128, S], F16, tag="av_sb")
                nc.vector.tensor_copy(out=av_sb, in_=av_ps)
                nc.gpsimd.dma_start(out=av_d.ap()[w], in_=av_sb)

            lepe_out(*prev_lp)

    _split_multiwaits(nc)
    return nc


# ------------------------------------------------------------------ host ----

_NC_CACHE = {}


def _get_nc():
    if "nc" not in _NC_CACHE:
        _NC_CACHE["nc"] = _build_nc()
    return _NC_CACHE["nc"]


def _host_prep(qkv, conv_w):
    """Build per-core input arrays (all f16)."""
    f16 = np.float16
    # [3, B, 4096, 128] -> window grids [3, B, y, sx, x, c]
    qkv_w = qkv.reshape(3, B, RES, NWIN_IMG, SPLIT, C)

    cores = []
    for core in range(N_CORES):
        bs = slice(core * IMGS_PER_CORE, (core + 1) * IMGS_PER_CORE)
        q = qkv_w[0, bs]   # [2, y, sx, x, c]
        k = qkv_w[1, bs]
        v = qkv_w[2, bs]

        # [c, img, sx, y, x] -> [128, NW*512]
        qT = np.ascontiguousarray(q.transpose(4, 0, 2, 1, 3)).reshape(C, NW * S)
        kT = np.ascontiguousarray(k.transpose(4, 0, 2, 1, 3)).reshape(C, NW * S)

        # v token-major: [yy, x, img, sx, jc, c] -> [128, NW*4*128]
        vt = v.reshape(IMGS_PER_CORE, NCHUNK, 16, NWIN_IMG, SPLIT, C)
        vtok = np.ascontiguousarray(vt.transpose(2, 4, 0, 3, 1, 5)).reshape(
            128, NW * NCHUNK * C
        )

        # vpad: [c, win, 662] with interior at 12 + y*10 + x
        vimg = np.ascontiguousarray(v.transpose(4, 0, 2, 1, 3))  # [c, img, sx, y, x]
        vpad = np.zeros((C, NW, PADN), dtype=f16)
        vpad_v = vpad[:, :, 1:661].reshape(C, NW, RES + 2, XP)
        vpad_v[:, :, 1:-1, 1:-1] = vimg.reshape(C, NW, RES, SPLIT)

        cores.append(
            {
                "qT": qT.astype(f16),
                "kT": kT.astype(f16),
                "vtok": vtok.astype(f16),
                "vpad": vpad.reshape(C, NW * PADN),
            }
        )

    w9 = conv_w.reshape(C, 9).astype(np.float32)
    diag = np.zeros((C, 9, C), dtype=np.float32)
    idx = np.arange(C)
    for t in range(9):
        diag[idx, t, idx] = w9[:, t]
    diag = diag.reshape(C, 9 * C).astype(f16)
    for m in cores:
        m["diag"] = diag
        m["wtap"] = w9
    return cores


def kernel(qkv, conv_w, conv_b):
    from concourse.bass_utils import run_bass_kernel_spmd

    qkv = np.asarray(qkv, dtype=np.float32)
    conv_w = np.asarray(conv_w, np.float32)
    conv_b = np.asarray(conv_b, np.float32)

    nc = _get_nc()
    in_maps = _host_prep(qkv, conv_w)

    res = run_bass_kernel_spmd(nc, in_maps, core_ids=list(range(N_CORES)))
    global LAST_RESULT
    LAST_RESULT = res

    outs = []
    for r in res.results:
        av = r["avT"].astype(np.float32)       # [16, 128, 512]
        den = r["denT"].astype(np.float32)     # [16, 4, 512]
        lp = r["lepeT"].astype(np.float32)     # [16, 128, 512]
        att = av.reshape(NW, HEADS, HD, S) / den.reshape(NW, HEADS, 1, S)
        o = att.reshape(NW, C, S) + lp + conv_b.astype(np.float32)[None, :, None]
        # [win, c, s] -> [img, y, x, c]
        o = o.reshape(IMGS_PER_CORE, NWIN_IMG, C, RES, SPLIT)
        o = o.transpose(0, 3, 1, 4, 2).reshape(IMGS_PER_CORE, RES, RES, C)
        outs.append(o)
    return np.concatenate(outs, axis=0)


LAST_RESULT = None
